# revision 21
# baseline (speedup 1.0000x reference)
"""Trainium2 Bass kernel for DiscriminativeLoss (segment_reduce).

Full inputs: embedding [8, 32, 65536] f32, seg_gt [8, 65536] i32 (labels 0..20,
0 = background).  Output: (var_loss, dist_loss, reg_loss) scalars.

Sharding: pure data parallel — batch b -> core b.  Each core computes, for its
sample:
  pass 1 (pixel-on-partition layout, fp8 embedding): per-label sums+counts
         [84,132] via one-hot matmuls accumulated in PSUM,
  pass 2 (channel-on-partition layout): per-pixel squared distance to own
         centroid via (I | -M) matmuls grouped 4-wide over PSUM banks so the
         ident/-M stationaries are loaded once per group, hinge, and the
         w-weighted global reduction where w_l = present_l / counts_l.
The tiny 21x21 centroid pairwise loss and final scalar assembly run on host
from the per-core [84,132] segment-sum matrix and [128] partial var sums.
"""

import os
import sys
from contextlib import ExitStack

import numpy as np

for _p in ("/opt/trn_rl_repo", "/root/.axon_site/_ro/trn_rl_repo"):
    if os.path.isdir(_p) and _p not in sys.path:
        sys.path.insert(0, _p)

import ml_dtypes

import concourse.bass as bass
import concourse.bacc as bacc
import concourse.tile as tile
from concourse import mybir
from concourse.bass_utils import run_bass_kernel_spmd

BF16 = ml_dtypes.bfloat16
FP8 = ml_dtypes.float8_e4m3

B, D, N = 8, 32, 65536
LP = 21          # label slots 0..20 (0 = background)
C = 4            # chunk count (channel-on-partition packing)
NC4 = N // C     # 16384 pixels per chunk
G = 128          # pass-1 tiles (512 px each)
A4 = 4           # pixels per partition per pass-1 tile
GW = 132         # pass-1 rhs cols per tile: 4 a-blocks of (32 emb + 1 ones)
T2 = 32          # pass-2 tiles (512 cols each)
UG = 4           # pass-2 tiles per PSUM-bank group
DELTA_V = 0.5
DELTA_D = 3.0

EMB4_FP8 = False     # channel-layout embedding in fp8 (extra DMA savings)

# const tensor column offsets (bf16 [128, CST_W])
OFF_IOTA_L = 0            # [128, 672]  l pattern, tiled x8 slabs
OFF_IOTA_COL = 672        # [128, 1]    p % 32
OFF_IDENT = 673           # [128, 128]  identity
OFF_SEL = 801             # [128, 84]   eye(84) selector
OFF_ONES_BD8 = 885        # [128, 256]  8 shifted block-diag ones variants
OFF_MASK = 1141           # [128, 1]    1 for rows c*32+l with 1<=l<=20
CST_W = 1142

F32 = mybir.dt.float32
BF = mybir.dt.bfloat16
F8 = mybir.dt.float8e4
U8 = mybir.dt.uint8
OP = mybir.AluOpType
AF = mybir.ActivationFunctionType

# one-hot build slab split: (engine, slab_idx) lists
OHT_SLABS = 16            # ohT: 16 slabs of 672 cols (8 g each)
OH4_SLABS = 16            # oh4: 16 slabs of 1024 cols
OHT_GPS = set(int(x) for x in os.environ.get('GPS_T', '10,11,12,13,14,15').split(',') if x != '')   # slabs built on gpsimd
OH4_GPS = set(int(x) for x in os.environ.get('GPS_4', '10,11,12,13,14,15').split(',') if x != '')


def build_nc():
    e4dt = F8 if EMB4_FP8 else BF
    nc = bacc.Bacc()
    embT_d = nc.dram_tensor("embT", [128, G * GW], F8, kind="ExternalInput")
    segR_d = nc.dram_tensor("segR", [128, G, A4], U8, kind="ExternalInput")
    emb4_d = nc.dram_tensor("emb4", [128, NC4], e4dt, kind="ExternalInput")
    seg4_d = nc.dram_tensor("seg4", [128, NC4], U8, kind="ExternalInput")
    cst_d = nc.dram_tensor("cst", [128, CST_W], BF, kind="ExternalInput")
    xout_d = nc.dram_tensor("xout", [84, GW], F32, kind="ExternalOutput")
    vout_d = nc.dram_tensor("vout", [1, 1], F32, kind="ExternalOutput")

    with ExitStack() as ctx:
        tc = ctx.enter_context(tile.TileContext(nc))
        big = ctx.enter_context(tc.tile_pool(name="big", bufs=1))
        sm = ctx.enter_context(tc.tile_pool(name="sm", bufs=1))
        sqp = ctx.enter_context(tc.tile_pool(name="sqp", bufs=4))
        ps = ctx.enter_context(tc.tile_pool(name="ps", bufs=1, space="PSUM"))
        psD = ctx.enter_context(tc.tile_pool(name="psD", bufs=1, space="PSUM"))

        # ---- input DMAs, chunked so consumers pipeline against arrivals ----
        # segR first (gates the one-hot build), embT next (gates pass-1),
        # cst after (first needed for the ACT warm + extract)
        segR = big.tile([128, G, A4], U8)
        nc.sync.dma_start(out=segR, in_=segR_d[:, :, :])
        embT = big.tile([128, G * GW], F8)
        for i in range(4):
            w = G * GW // 4
            nc.sync.dma_start(out=embT[:, i * w:(i + 1) * w],
                              in_=embT_d[:, i * w:(i + 1) * w])
        cst = big.tile([128, CST_W], BF)
        nc.sync.dma_start(out=cst, in_=cst_d[:, :])

        # warm the ACT table with a Sqrt first so the (only) table set loaded
        # is sqrt_and_others, which also contains square/relu/copy -> no
        # mid-kernel ACT_TABLE_LOAD.  zbias doubles as the all-zero f32 bias.
        zbias = sm.tile([128, 1], F32)
        nc.scalar.activation(zbias, cst[:, 0:1], AF.Sqrt, bias=0.0, scale=0.0)

        sel32 = big.tile([128, 84], F32)
        nc.vector.tensor_copy(sel32, cst[:, OFF_SEL:OFF_SEL + 84])
        icb32 = sm.tile([128, 1], F32)
        nc.vector.tensor_copy(icb32, cst[:, OFF_IOTA_COL:OFF_IOTA_COL + 1])
        # pass-2 feeds: emb4 chunk pairs with the matching seg4 chunk behind
        seg4 = big.tile([128, NC4], U8)
        emb4 = big.tile([128, NC4], e4dt)
        we, ws = NC4 // 8, NC4 // 4
        for j in range(4):
            nc.sync.dma_start(out=emb4[:, 2 * j * we:(2 * j + 1) * we],
                              in_=emb4_d[:, 2 * j * we:(2 * j + 1) * we])
            nc.sync.dma_start(out=emb4[:, (2 * j + 1) * we:(2 * j + 2) * we],
                              in_=emb4_d[:, (2 * j + 1) * we:(2 * j + 2) * we])
            nc.sync.dma_start(out=seg4[:, j * ws:(j + 1) * ws],
                              in_=seg4_d[:, j * ws:(j + 1) * ws])

        # stationaries for the extract scatter, zeroed off the critical path
        lhsT_OH = sm.tile([128, 128], BF)
        nc.vector.memset(lhsT_OH, 0.0)
        lhsT_W1 = sm.tile([128, 4], BF)
        nc.vector.memset(lhsT_W1, 0.0)
        lhsT_W8 = sm.tile([128, 256], BF)
        nc.vector.memset(lhsT_W8, 0.0)
        ones1 = sm.tile([128, 1], F32)
        nc.vector.memset(ones1, 1.0)

        # one-hot, pixel-on-partition: ohT[p, g, l*4+a] = (seg[p,g,a] == l)
        # (g-innermost would enable a faster DVE mode but makes the lhsT
        #  columns strided, which kills FWL and slows LDWEIGHTS by ~12us)
        ohT = big.tile([128, G, 84], BF)
        osc = sm.tile([128, G // 2, A4], F32)
        biasN = sm.tile([128, 7], F32)
        for k in range(6):
            nc.vector.memset(biasN[:, k:k + 1], float(-(15 + k)))
        nc.vector.memset(biasN[:, 6:7], 1.0)
        for h in range(2):
            gs = slice(h * (G // 2), (h + 1) * (G // 2))
            for l in range(LP):
                if l < 15:
                    nc.vector.tensor_scalar(
                        out=ohT[:, gs, l * A4:(l + 1) * A4],
                        in0=segR[:, gs, :], scalar1=float(l), scalar2=None,
                        op0=OP.is_equal)
                else:
                    # one-hot via ACT: relu(1 - (seg - l)^2) is exact on ints
                    nc.scalar.activation(osc, segR[:, gs, :], AF.Square,
                                         bias=biasN[:, l - 15:l - 14],
                                         scale=1.0)
                    nc.scalar.activation(ohT[:, gs, l * A4:(l + 1) * A4],
                                         osc, AF.Relu, bias=biasN[:, 6:7],
                                         scale=-1.0)

        # ---- pass 1: X[(a,l), (a',(d|1))] = sum_p ohT * embT ----
        X_ps = ps.tile([84, GW], F32)
        for g in range(G):
            nc.tensor.matmul(
                X_ps,
                lhsT=ohT[:, g, :],
                rhs=embT[:, g * GW:(g + 1) * GW],
                start=(g == 0), stop=(g == G - 1))
        Xs = sm.tile([84, GW], F32)
        nc.vector.tensor_copy(Xs, X_ps)
        nc.sync.dma_start(out=xout_d[:, :], in_=Xs)

        # ---- extract: sums+counts -> -means (bf16), w (f32), vectorized ----
        # M_ps[cb*32+l, 0:32] = sums, [.,32] = counts (diag-in-a reduction)
        M_ps = ps.tile([128, 33], F32)
        for cb in range(4):
            for a in range(A4):
                nc.tensor.matmul(
                    M_ps[cb * 32:cb * 32 + 21, :],
                    lhsT=sel32[0:84, a * 21:(a + 1) * 21],
                    rhs=Xs[:, a * 33:(a + 1) * 33],
                    start=(a == 0), stop=(a == 3),
                    tile_position=(0, cb * 32), skip_group_check=True)

        # keep the PE array active through the extract window so HAM does
        # not re-throttle before pass 2 (X_ps is dead after the Xs copy)
        for wrm in range(8):
            nc.tensor.matmul(X_ps, lhsT=ohT[:, wrm, :],
                             rhs=embT[:, :GW], start=True, stop=True,
                             skip_group_check=True)

        with tc.high_priority():
            cnt = sm.tile([128, 1], F32)
            nc.vector.tensor_scalar(out=cnt, in0=M_ps[:, 32:33], scalar1=1.0,
                                    scalar2=None, op0=OP.max)
            rec = sm.tile([128, 1], F32)
            nc.vector.reciprocal(rec, cnt)
            pres = sm.tile([128, 1], F32)
            nc.vector.tensor_scalar(out=pres, in0=M_ps[:, 32:33], scalar1=0.0,
                                    scalar2=None, op0=OP.is_gt)
            # w = pres * mask * (1/cnt)
            w1 = sm.tile([128, 1], F32)
            nc.vector.scalar_tensor_tensor(
                out=w1, in0=pres, scalar=cst[:, OFF_MASK:OFF_MASK + 1],
                in1=rec, op0=OP.mult, op1=OP.mult)
            # -mean = sums * (-1) * (1/cnt)
            nmu = sm.tile([128, 32], BF)
            nc.vector.scalar_tensor_tensor(
                out=nmu, in0=M_ps[:, 0:32], scalar=-1.0,
                in1=rec.to_broadcast((128, 32)), op0=OP.mult, op1=OP.mult)
            for cb in range(4):
                sl = slice(cb * 32, cb * 32 + 21)
                if cb % 2 == 0:
                    nc.vector.tensor_copy(lhsT_OH[sl, cb * 32:(cb + 1) * 32],
                                          nmu[sl])
                    nc.vector.tensor_copy(lhsT_W1[sl, cb:cb + 1], w1[sl])
                else:
                    nc.scalar.copy(lhsT_OH[sl, cb * 32:(cb + 1) * 32],
                                   nmu[sl])
                    nc.scalar.copy(lhsT_W1[sl, cb:cb + 1], w1[sl])
            for u in range(8):
                o = u * 32 + u * 4
                eng = nc.vector if u % 2 == 0 else nc.scalar
                if eng is nc.scalar:
                    nc.scalar.copy(lhsT_W8[:, o:o + 4], lhsT_W1)
                else:
                    nc.vector.tensor_copy(lhsT_W8[:, o:o + 4], lhsT_W1)

        # one-hot, label-on-partition: oh4[c*32+l, m] = (seg[c*16384+m] == l)
        # (emitted after the extract chain so the tiny critical-path DVE ops
        #  aren't queued behind these big slabs)
        # single-src tensor_scalar (per-partition compare target) so the DVE
        # can run a 2-port perf mode instead of 1x scalar_tensor_tensor
        oh4 = big.tile([128, NC4], BF)
        for s in range(OH4_SLABS):
            sl = slice(s * 1024, (s + 1) * 1024)
            with tc.tile_wait_until(0.024 + s * 0.0008):
                nc.vector.tensor_scalar(
                    out=oh4[:, sl], in0=seg4[:, sl], scalar1=icb32,
                    scalar2=None, op0=OP.is_equal)

        # ---- pass 2, grouped so PSUM banks rotate 4-wide ----
        # D matmuls stay full-array: HAM does not count 32x32 tile matmuls
        # as PE activity, so a tiled pass-2 runs at the cold clock.
        A_ps = ps.tile([128, 512], F32)   # per-pixel |e - mu|^2
        B_ps = ps.tile([128, 512], F32)   # per-pixel w
        ident = cst[:, OFF_IDENT:OFF_IDENT + 128]

        def emit_A(t, sqt):
            Tt, ut = t // 8, t % 8
            nc.tensor.matmul(
                A_ps[Tt * 32:(Tt + 1) * 32, :],
                lhsT=cst[:, OFF_ONES_BD8 + ut * 32:
                         OFF_ONES_BD8 + (ut + 1) * 32],
                rhs=sqt, start=(t % 8 == 0), stop=(t % 8 == 7),
                tile_position=(0, Tt * 32), skip_group_check=True)

        # the A matmul for tile t is emitted one tile late so the PE never
        # waits on ACT's Square of its own tile
        pend = None
        for grp in range(T2 // UG):
            banks = [psD.tile([128, 512], F32, name=f"D{u}")
                     for u in range(UG)]
            cols = [slice((grp * UG + u) * 512, (grp * UG + u + 1) * 512)
                    for u in range(UG)]
            for u in range(UG):
                nc.tensor.matmul(banks[u], lhsT=ident, rhs=emb4[:, cols[u]],
                                 start=True, stop=False, skip_group_check=True)
                nc.tensor.matmul(banks[u], lhsT=lhsT_OH, rhs=oh4[:, cols[u]],
                                 start=False, stop=True, skip_group_check=True)
            for u in range(UG):
                t = grp * UG + u
                Tt, ut = t // 8, t % 8
                sqt = sqp.tile([128, 512], BF)
                nc.scalar.activation(sqt, banks[u], AF.Square,
                                     bias=zbias[:, 0:1])
                nc.tensor.matmul(
                    B_ps[Tt * 32:(Tt + 1) * 32, :],
                    lhsT=lhsT_W8[:, ut * 32:(ut + 1) * 32],
                    rhs=oh4[:, cols[u]], start=(t % 8 == 0), stop=(t % 8 == 7),
                    tile_position=(0, Tt * 32), skip_group_check=True)
                if pend is not None:
                    emit_A(*pend)
                pend = (t, sqt)
        emit_A(*pend)

        # tail: d = sqrt(A); r = max(d - dv, 0); vn = sum(r*r*B), split in
        # partition halves so the first half overlaps pass-2's second half
        vn = sm.tile([128, 1], F32)
        d_sb = sm.tile([128, 512], F32)
        r_sb = sm.tile([128, 512], F32)
        rw_sb = sm.tile([128, 512], F32)
        vw = sm.tile([128, 512], F32)
        for th in range(2):
            q = slice(th * 64, (th + 1) * 64)
            nc.scalar.activation(d_sb[q], A_ps[q], AF.Sqrt, bias=zbias[q, 0:1])
            nc.vector.tensor_scalar(out=r_sb[q], in0=d_sb[q],
                                    scalar1=-DELTA_V, scalar2=0.0,
                                    op0=OP.add, op1=OP.max)
            nc.vector.scalar_tensor_tensor(
                out=rw_sb[q], in0=r_sb[q], scalar=0.0, in1=B_ps[q],
                op0=OP.add, op1=OP.mult)
            nc.vector.scalar_tensor_tensor(
                out=vw[q], in0=rw_sb[q], scalar=0.0, in1=r_sb[q],
                op0=OP.add, op1=OP.mult, accum_out=vn[q])
        # reduce the per-partition partials to one scalar so the final DMA
        # is a single-descriptor 4-byte write (16-engine sem-inc tail cost)
        nc.tensor.matmul(M_ps[0:1, 0:1], lhsT=ones1, rhs=vn,
                         start=True, stop=True, skip_group_check=True)
        vs_sb = sm.tile([1, 1], F32)
        nc.vector.tensor_copy(vs_sb, M_ps[0:1, 0:1])
        nc.sync.dma_start(out=vout_d[:, :], in_=vs_sb)

    nc.compile()
    return nc


def _make_consts():
    cst = np.zeros((128, CST_W), np.float32)
    iota_l = np.tile(np.arange(LP), A4)          # [84]
    cst[:, OFF_IOTA_L:OFF_IOTA_L + 672] = np.tile(iota_l, 8)[None, :]
    cst[:, OFF_IOTA_COL] = np.arange(128) % 32
    cst[:, OFF_IDENT:OFF_IDENT + 128] = np.eye(128)
    sel = np.zeros((84, 84), np.float32)     # rows (l,a)=l*4+a, col a*21+l
    for l in range(LP):
        for a in range(A4):
            sel[l * A4 + a, a * LP + l] = 1.0
    cst[0:84, OFF_SEL:OFF_SEL + 84] = sel
    ones8 = np.zeros((128, 8, 32), np.float32)
    for c in range(C):
        for d in range(32):
            for u in range(8):
                ones8[c * 32 + d, u, u * 4 + c] = 1.0
    cst[:, OFF_ONES_BD8:OFF_ONES_BD8 + 256] = ones8.reshape(128, 256)
    mask = np.zeros(128, np.float32)
    for c in range(C):
        mask[c * 32 + 1:c * 32 + LP] = 1.0
    cst[:, OFF_MASK] = mask
    return cst.astype(BF16)


def _prep_core(emb_b, seg_b, cst):
    """emb_b [32, 65536] f32, seg_b [65536] i32 -> per-core input map."""
    Tm = np.ascontiguousarray(emb_b.T)                       # [N, 32]
    t4 = Tm.reshape(G, 128, A4, 32).transpose(1, 0, 2, 3)    # [p, g, a, d]
    embT = np.empty((128, G, A4, 33), FP8)
    embT[:, :, :, :32] = t4.astype(FP8)
    embT[:, :, :, 32] = FP8(1.0)
    s4 = seg_b.reshape(G, 128, A4).transpose(1, 0, 2)        # [p, g, a]
    segR = np.ascontiguousarray(s4).reshape(128, G, A4).astype(np.uint8)
    emb4 = np.ascontiguousarray(
        emb_b.reshape(32, C, NC4).transpose(1, 0, 2)).reshape(128, NC4)
    seg4 = np.ascontiguousarray(
        np.broadcast_to(seg_b.reshape(C, 1, NC4), (C, 32, NC4))
    ).reshape(128, NC4).astype(np.uint8)
    return {
        "embT": embT.reshape(128, G * GW),
        "segR": segR,
        "emb4": emb4.astype(FP8 if EMB4_FP8 else BF16),
        "seg4": seg4,
        "cst": cst,
    }


_NC_CACHE = None


def _get_nc():
    global _NC_CACHE
    if _NC_CACHE is None:
        _NC_CACHE = build_nc()
    return _NC_CACHE


def _host_finish(X, vn):
    """X [84, 132] f32 (pass-1 matrix), vn [128, 1] f32 -> (var_b, dist_b)."""
    Xr = X.reshape(LP, A4, GW).astype(np.float64)
    counts = np.zeros(LP)
    sums = np.zeros((LP, 32))
    for a in range(A4):
        sums += Xr[:, a, a * 33:a * 33 + 32]
        counts += Xr[:, a, a * 33 + 32]
    means = sums / np.maximum(counts, 1.0)[:, None]
    pres = counts > 0
    pres[0] = False
    nl = float(pres.sum())
    var_b = float(vn.sum()) / max(nl, 1.0) if nl > 0 else 0.0
    m = means[1:]
    p = pres[1:]
    sqd = ((m[:, None, :] - m[None, :, :]) ** 2).sum(-1)
    dist = np.sqrt(np.maximum(sqd, 0.0))
    pair = (p[:, None] & p[None, :]) & ~np.eye(LP - 1, dtype=bool)
    dl = (np.maximum(DELTA_D - dist, 0.0) ** 2 * pair).sum()
    denom = max(nl * (nl - 1.0), 1.0)
    dist_b = dl / denom / 2.0 if nl > 1 else 0.0
    return var_b, dist_b


def kernel(embedding, seg_gt):
    embedding = np.asarray(embedding, np.float32)
    seg_gt = np.asarray(seg_gt, np.int32)
    cst = _make_consts()
    in_maps = [_prep_core(embedding[b], seg_gt[b], cst) for b in range(B)]
    nc = _get_nc()
    res = run_bass_kernel_spmd(nc, in_maps, core_ids=list(range(B)))
    var_l, dist_l = [], []
    for b in range(B):
        var_b, dist_b = _host_finish(res.results[b]["xout"],
                                     res.results[b]["vout"])
        var_l.append(var_b)
        dist_l.append(dist_b)
    return (np.float32(np.mean(var_l)), np.float32(np.mean(dist_l)),
            np.float32(0.0))
